# revision 4
# baseline (speedup 1.0000x reference)
"""BrahmanAttention Trainium2 kernel, v3 (bf16 data path, tuned schedule).

Multi-head attention with a per-head case-pair bias (gathered via one-hot
augmentation of the QK contraction) and a per-head verb-position bias
(folded into the exp activation as a per-partition bias).

Sharding: core c = (batch b = c//2, head-half g = c%2). Each of the 8
NeuronCores computes one batch x 8 heads. Wq/Wk/Wv are column-sharded and Wo
row-sharded by head group, so each core emits a partial [L, D] output; the
host sums the two partials per batch and adds the constant row bv @ Wo + bo.

v3 structure:
  - every DMA'd tensor is fp16 (halves HBM + host-transfer bytes; matmul
    rate is unchanged per the TRN2 cost model at free-dim >= 256); the
    output partial is bf16 too (host accumulates in f32)
  - merged DMAs (one per weight matrix, wq/wk split in two halves so the
    first Q/K projections can start while the rest streams in)
  - Q/K pair-0 projections first; V-projection blocks and later-pair
    projections run as fillers inside the attention head loops
  - persistent qat/kat tiles: one-hot / case-bias augmentation rows are
    DMA'd once, only the 0:64 rows are rewritten per pair
  - warm-up matmuls keep the PE p-state at full clock through the initial
    DMA window
  - PSUM: 2 proj banks + 3 score banks + 1 rz-broadcast bank + 2 AV banks
  - engines: exp on ACT; evacuations split DVE/gpsimd; reciprocal + 1/Z
    multiply on DVE; V-staging on gpsimd
"""

import sys

if "/opt/trn_rl_repo" not in sys.path:
    sys.path.insert(0, "/opt/trn_rl_repo")

import numpy as np

B, L, D, H = 4, 1024, 1024, 16
HD = D // H            # 64
NUM_CASES = 8
SCALE = 8.0            # sqrt(HD)
HPC = 8                # heads per core
DHC = HPC * HD         # 512 head-dims per core
NCORES = 8
KAUG = HD + NUM_CASES  # 72 augmented contraction dim

_cached = {}


def _build_nc():
    import concourse.bass as bass
    import concourse.tile as tile
    from concourse import bacc, mybir
    from contextlib import ExitStack

    f32 = mybir.dt.float32
    f32r = mybir.dt.float32r
    bf16 = mybir.dt.float16
    Exp = mybir.ActivationFunctionType.Exp

    nc = bacc.Bacc("TRN2", target_bir_lowering=False, debug=False,
                   num_devices=NCORES)

    xt_d = nc.dram_tensor("xt", [D, L], bf16, kind="ExternalInput")
    wq_d = nc.dram_tensor("wq", [D, DHC], bf16, kind="ExternalInput")
    wk_d = nc.dram_tensor("wk", [D, DHC], bf16, kind="ExternalInput")
    wv_d = nc.dram_tensor("wv", [D, DHC], bf16, kind="ExternalInput")
    wo_d = nc.dram_tensor("wo", [DHC, D], bf16, kind="ExternalInput")
    eoh_d = nc.dram_tensor("eoh", [NUM_CASES, L], bf16, kind="ExternalInput")
    ksg_d = nc.dram_tensor("ksg", [HPC * NUM_CASES, L], bf16,
                           kind="ExternalInput")
    vbh_d = nc.dram_tensor("vbh", [128, HPC * 8], f32, kind="ExternalInput")
    bq_d = nc.dram_tensor("bq", [1, DHC], f32r, kind="ExternalInput")
    ones_d = nc.dram_tensor("onesr", [1, L], f32r, kind="ExternalInput")
    onesh_d = nc.dram_tensor("onesh", [1, L], bf16, kind="ExternalInput")
    out_d = nc.dram_tensor("out", [L, D], bf16, kind="ExternalOutput")

    def mm(out, lhsT, rhs, start, stop):
        nc.tensor.matmul(out, lhsT, rhs, start=start, stop=stop)

    with tile.TileContext(nc) as tc, ExitStack() as ctx:
        pp = ctx.enter_context(tc.tile_pool(name="persist", bufs=1))

        # single [128, 8*520] va tile: per jb, 8 heads x (64 V cols | 1 one)
        va = pp.tile([128, 8, HPC * (HD + 1)], bf16, name="va", tag="va")
        otf = [pp.tile([128, L], bf16, name=f"otf{dc}", tag=f"otf{dc}")
               for dc in range(4)]
        vb_sb = pp.tile([128, HPC, 8], f32, name="vb", tag="vb")
        bq_sb = pp.tile([1, DHC], f32r, name="bq", tag="bq")
        ones_row = pp.tile([1, 512], f32r, name="ones", tag="ones")
        xt = pp.tile([128, 8, L], bf16, name="xt", tag="xt")
        wq_sb = pp.tile([128, 8, DHC], bf16, name="wq", tag="wq")
        wk_sb = pp.tile([128, 8, DHC], bf16, name="wk", tag="wk")
        wv_sb = pp.tile([128, 8, DHC], bf16, name="wv", tag="wv")
        wo_sb = pp.tile([128, 4, D], bf16, name="wo", tag="wo")
        # persistent per-head augmented Q^T / K^T tiles
        qat = [pp.tile([KAUG, L], bf16, name=f"qat{h}", tag=f"qat{h}")
               for h in range(8)]
        kat = [pp.tile([KAUG, L], bf16, name=f"kat{h}", tag=f"kat{h}")
               for h in range(8)]

        def big3(dst, src_d, nchunk, rows_per, cols):
            """one DMA: DRAM [nchunk*rows_per, cols] -> SBUF [rows_per, nchunk, cols]"""
            nc.sync.dma_start(
                out=dst,
                in_=bass.AP(tensor=src_d[:, :].tensor, offset=0,
                            ap=[[cols, rows_per], [rows_per * cols, nchunk],
                                [1, cols]]))

        # pair-0 aug rows first so head 0 can start early.
        for h in (0, 1):
            nc.sync.dma_start(out=qat[h][HD:KAUG, :], in_=eoh_d[:, :])
            nc.sync.dma_start(out=kat[h][HD:KAUG, :],
                              in_=ksg_d[h * 8:h * 8 + 8, :])
        big3(xt, xt_d, 8, 128, L)
        nc.sync.dma_start(out=bq_sb, in_=bq_d[:, :])
        for hf in range(2):   # wq, wk in halves so projections can stream
            nc.sync.dma_start(
                out=wq_sb[:, 4 * hf:4 * hf + 4, :],
                in_=bass.AP(tensor=wq_d[:, :].tensor, offset=hf * 4 * 128 * DHC,
                            ap=[[DHC, 128], [128 * DHC, 4], [1, DHC]]))
        for hf in range(2):
            nc.sync.dma_start(
                out=wk_sb[:, 4 * hf:4 * hf + 4, :],
                in_=bass.AP(tensor=wk_d[:, :].tensor, offset=hf * 4 * 128 * DHC,
                            ap=[[DHC, 128], [128 * DHC, 4], [1, DHC]]))
        nc.sync.dma_start(out=vb_sb,
                          in_=vbh_d[:, :].rearrange("p (h jb) -> p h jb", jb=8))
        nc.sync.dma_start(out=ones_row, in_=ones_d[:, 0:512])
        big3(wv_sb, wv_d, 8, 128, DHC)
        # all 64 ones-columns of va in one stride-65 DMA
        nc.sync.dma_start(
            out=va.rearrange("p jb (h c) -> p (jb h) c",
                             c=HD + 1)[:, :, HD:HD + 1],
            in_=bass.AP(tensor=onesh_d[:, :].tensor, offset=0,
                        ap=[[0, 128], [1, 8 * HPC], [1, 1]]))
        for h in range(2, 8):
            nc.sync.dma_start(out=qat[h][HD:KAUG, :], in_=eoh_d[:, :])
            nc.sync.dma_start(out=kat[h][HD:KAUG, :],
                              in_=ksg_d[h * 8:h * 8 + 8, :])
        big3(wo_sb, wo_d, 4, 128, D)

        with tc.tile_pool(name="stage", bufs=2) as sp, \
             tc.tile_pool(name="attn", bufs=4) as ap_, \
             tc.tile_pool(name="attn1", bufs=1) as a1, \
             tc.tile_pool(name="pps", bufs=2, space="PSUM") as pps, \
             tc.tile_pool(name="sps", bufs=3, space="PSUM") as sps, \
             tc.tile_pool(name="rzp", bufs=1, space="PSUM") as rzp, \
             tc.tile_pool(name="otps", bufs=1, space="PSUM") as otp:

            # PE warm-up: junk matmuls on eoh keep the p-state at full clock
            # while the weight DMAs stream in (never read back).
            warm = rzp.tile([128, 512], f32, name="warm", tag="rz")

            def warmup(n):
                for w in range(n):
                    mm(warm, qat[0][HD:KAUG, 0:128], qat[0][HD:KAUG, 0:512],
                       True, True)

            warmup(24)

            def vproj(jb):
                """V: one [128 j, 512 dh] block accumulated over e; scatter
                into va with stride 65 (ones column interleaved)"""
                jsl = slice(jb * 128, jb * 128 + 128)
                ps = pps.tile([128, DHC], f32, name="pp", tag="pp")
                for e in range(8):
                    mm(ps, xt[:, e, jsl], wv_sb[:, e, :], e == 0, e == 7)
                st = sp.tile([128, DHC], bf16, name="vst", tag="vst")
                nc.scalar.copy(st, ps)
                nc.sync.dma_start(
                    out=va[:, jb, :].rearrange("p (h c) -> p h c",
                                               c=HD + 1)[:, :, 0:HD],
                    in_=st.rearrange("p (h c) -> p h c", c=HD))

            def proj_half(w_sb, dhb, ih, dst_pair, bq_slice, evac):
                """one [128, 512] psum group of a pair projection; evac=True
                writes both evacuation copies for this ih."""
                isl = slice(ih * 512, ih * 512 + 512)
                ps = pps.tile([128, 512], f32, name="pp", tag="pp")
                for e in range(8):
                    mm(ps, w_sb[:, e, 128 * dhb:128 * dhb + 128],
                       xt[:, e, isl], e == 0,
                       (e == 7) and bq_slice is None)
                if bq_slice is not None:  # + bq/SCALE (exact bq handling)
                    mm(ps, bq_slice, ones_row[:, :], False, True)
                for half in range(2):
                    nc.vector.tensor_copy(dst_pair[half][0:HD, isl],
                                          ps[64 * half:64 * half + 64, :])

            def head(h, deferred, fillers):
                """attention for one head; returns a deferred-tail closure.
                fillers: list of zero-arg closures, one run per jb step, to
                interleave independent PE work."""
                qat_h, kat_h = qat[h], kat[h]
                ot_ps = otp.tile([HD + 1, L], f32, name="ot", tag="ot")
                pend = {}
                for jb in range(8):
                    jsl = slice(jb * 128, jb * 128 + 128)
                    es = ap_.tile([128, L], bf16, name="es", tag="es")
                    for ih in range(2):   # ACT cannot read across PSUM banks
                        isl = slice(ih * 512, ih * 512 + 512)
                        s_ps = sps.tile([128, 512], f32, name="s", tag="s")
                        mm(s_ps, kat_h[:, jsl], qat_h[:, isl], True, True)
                        nc.scalar.activation(es[:, isl], s_ps, Exp,
                                             bias=vb_sb[:, h, jb:jb + 1])
                    pend[jb] = es
                    if fillers:
                        fillers.pop(0)()
                    if jb == 2 and deferred is not None:
                        deferred()   # previous head's rzb matmul + normalize
                    if jb >= 3:
                        av(h, jb - 3, ot_ps, pend.pop(jb - 3))
                for jb in (5, 6, 7):
                    av(h, jb, ot_ps, pend.pop(jb))

                rz1 = a1.tile([1, L], f32r, name="rz1", tag="rz1")
                with nc.allow_low_precision(reason="f32r keeps ~17 mantissa "
                                            "bits; 1/Z tolerates it"):
                    nc.vector.reciprocal(rz1, ot_ps[HD:HD + 1, :])

                def tail():
                    # broadcast 1/Z across 64 partitions via a K=1 matmul
                    # (fp32r rounds at ~2^-17 — negligible here)
                    for ih in range(2):
                        isl = slice(ih * 512, ih * 512 + 512)
                        rzb_ps = rzp.tile([HD, 512], f32, name="rzb", tag="rz")
                        mm(rzb_ps, ones_row[:, 0:HD], rz1[:, isl], True, True)
                        rzb = a1.tile([HD, 512], f32, name=f"rzb{ih}",
                                      tag=f"rzb{ih}")
                        nc.vector.tensor_copy(rzb, rzb_ps)
                        nc.vector.tensor_mul(
                            otf[h // 2][64 * (h % 2):64 * (h % 2) + 64, isl],
                            ot_ps[0:HD, isl], rzb)
                return tail

            def av(h, jb, ot_ps, es):
                lh = va[:, jb, h * (HD + 1):(h + 1) * (HD + 1)]
                for ih in range(2):
                    isl = slice(ih * 512, ih * 512 + 512)
                    mm(ot_ps[:, isl], lh, es[:, isl], jb == 0, jb == 7)

            # filler schedule: head 0 runs the 8 V-projection blocks; head
            # 2p+1 runs the 4 projection units of pair p+1.
            fill = {0: [lambda jb=jb: vproj(jb) for jb in range(8)]}
            for p in (1, 2, 3):
                units = []
                for isbq in (True, False):
                    for ih in range(2):
                        units.append(lambda ih=ih, p=p, isbq=isbq:
                                     proj_half(
                                         wq_sb if isbq else wk_sb, p, ih,
                                         [qat[2 * p] if isbq else kat[2 * p],
                                          qat[2 * p + 1] if isbq else kat[2 * p + 1]],
                                         bq_sb[:, 128 * p:128 * p + 128]
                                         if isbq else None,
                                         True))
                fill[2 * p - 1] = units

            deferred = None
            for ih in range(2):
                proj_half(wq_sb, 0, ih, [qat[0], qat[1]],
                          bq_sb[:, 0:128], True)
            warmup(3)
            for ih in range(2):
                proj_half(wk_sb, 0, ih, [kat[0], kat[1]], None, True)
            for h in range(8):
                deferred = head(h, deferred, fill.get(h, []))
            deferred()  # last head's tail

            # ---- output projection ----------------------------------------
            with tc.tile_pool(name="fin", bufs=3) as fp:
                for ib in range(8):
                    isl = slice(ib * 128, ib * 128 + 128)
                    osb = fp.tile([128, D], bf16, name="osb", tag="osb")
                    for eh in range(2):
                        esl = slice(eh * 512, eh * 512 + 512)
                        f_ps = pps.tile([128, 512], f32, name="pp", tag="pp")
                        for dc in range(4):
                            mm(f_ps, otf[dc][:, isl], wo_sb[:, dc, esl],
                               dc == 0, dc == 3)
                        if (ib + eh) % 2:
                            nc.vector.tensor_copy(osb[:, esl], f_ps)
                        else:
                            nc.scalar.copy(osb[:, esl], f_ps)
                    nc.sync.dma_start(out=out_d[isl, :], in_=osb)

    nc.compile()
    return nc


def _get_nc():
    if "nc" not in _cached:
        _cached["nc"] = _build_nc()
    return _cached["nc"]


def make_in_maps(**inputs):
    """Host-side sharding: returns (in_maps for cores 0..7, co row [D])."""
    bf16 = np.float16

    x = np.asarray(inputs["x"], np.float32)
    case_ids = np.asarray(inputs["case_ids"])
    verb_mask = np.asarray(inputs["verb_mask"])
    Wq = np.asarray(inputs["Wq"], np.float32)
    bq = np.asarray(inputs["bq"], np.float32)
    Wk = np.asarray(inputs["Wk"], np.float32)
    Wv = np.asarray(inputs["Wv"], np.float32)
    Wo = np.asarray(inputs["Wo"], np.float32)
    bo = np.asarray(inputs["bo"], np.float32)
    bv = np.asarray(inputs["bv"], np.float32)
    case_bias = np.asarray(inputs["case_bias"], np.float32)
    verb_bias = np.asarray(inputs["verb_bias"], np.float32)
    # NOTE: bk is exactly absorbed by softmax shift invariance; bv/bo are
    # added on the host as co = bv @ Wo + bo (attention rows sum to 1).
    co = (bv @ Wo + bo).astype(np.float32)

    wq_h = (Wq / SCALE).astype(bf16)
    wk_h = Wk.astype(bf16)
    wv_h = Wv.astype(bf16)
    wo_h = Wo.astype(bf16)
    ones_f = np.ones((1, L), np.float32)
    ones_h = np.ones((1, L), bf16)

    in_maps = []
    for c in range(NCORES):
        b, g = c // 2, c % 2
        cols = slice(g * DHC, (g + 1) * DHC)
        hs = np.arange(g * HPC, (g + 1) * HPC)
        eoh = (case_ids[b][None, :] == np.arange(NUM_CASES)[:, None])
        in_maps.append({
            "xt": np.ascontiguousarray(x[b].T).astype(bf16),
            "wq": np.ascontiguousarray(wq_h[:, cols]),
            "wk": np.ascontiguousarray(wk_h[:, cols]),
            "wv": np.ascontiguousarray(wv_h[:, cols]),
            "wo": np.ascontiguousarray(wo_h[cols, :]),
            "eoh": eoh.astype(bf16),
            # ksg rows h*8+c over j: case_bias[h][c, case_j]
            "ksg": np.ascontiguousarray(
                (case_bias[hs] @ eoh.astype(np.float32)).reshape(
                    HPC * NUM_CASES, L)).astype(bf16),
            # [128 p, (h jb)]: entry = verb_bias[h] * verb_mask[b, jb*128+p]
            "vbh": np.ascontiguousarray(
                (verb_bias[hs][None, :, None] *
                 verb_mask[b].reshape(8, 128).T[:, None, :])
                .reshape(128, HPC * 8)).astype(np.float32),
            "bq": np.ascontiguousarray(bq[None, cols] / SCALE),
            "onesr": ones_f,
            "onesh": ones_h,
        })
    return in_maps, co


def gather(results, co):
    out = np.empty((B, L, D), np.float32)
    for b in range(B):
        out[b] = (results[2 * b]["out"].astype(np.float32) +
                  results[2 * b + 1]["out"].astype(np.float32) + co)
    return out


def _get_runner():
    """Build (once) a cached jitted SPMD executor over the 8 cores.

    run_bass_kernel_spmd re-traces and re-jits on every call (~1.5s); this
    caches the compiled executable so repeated kernel() calls only pay
    host-side sharding + transfer + execute.
    """
    if "runner" in _cached:
        return _cached["runner"]

    import jax
    from jax.experimental.shard_map import shard_map
    from jax.sharding import Mesh, PartitionSpec
    from concourse import bass2jax, mybir

    nc = _get_nc()
    bass2jax.install_neuronx_cc_hook()
    partition_name = (nc.partition_id_tensor.name
                      if nc.partition_id_tensor else None)

    in_names, out_names, out_avals, zero_outs = [], [], [], []
    for alloc in nc.m.functions[0].allocations:
        if not isinstance(alloc, mybir.MemoryLocationSet):
            continue
        name = alloc.memorylocations[0].name
        if alloc.kind == "ExternalInput":
            if name != partition_name:
                in_names.append(name)
        elif alloc.kind == "ExternalOutput":
            out_names.append(name)
            shape = tuple(alloc.tensor_shape)
            dtype = mybir.dt.np(alloc.dtype)
            out_avals.append(jax.core.ShapedArray(shape, dtype))
            zero_outs.append(np.zeros((NCORES * shape[0],) + shape[1:],
                                      dtype))
    n_params, n_outs = len(in_names), len(out_avals)
    in_names_all = (in_names + out_names +
                    ([partition_name] if partition_name else []))

    def _body(*args):
        ins = list(args[:n_params])
        outs = list(args[n_params:n_params + n_outs])
        pid = [bass2jax.partition_id_tensor()] if partition_name else []
        outs = list(bass2jax._bass_exec_p.bind(
            *(ins + outs + pid), out_avals=tuple(out_avals),
            in_names=tuple(in_names_all), out_names=tuple(out_names),
            lowering_input_output_aliases=(), sim_require_finite=True,
            sim_require_nnan=True, nc=nc))
        return tuple(outs)

    mesh = Mesh(np.asarray(jax.devices()[:NCORES]), ("core",))
    fn = jax.jit(shard_map(_body, mesh=mesh,
                           in_specs=(PartitionSpec("core"),) * (n_params + n_outs),
                           out_specs=(PartitionSpec("core"),) * n_outs,
                           check_rep=False), keep_unused=True)
    runner = {"fn": fn, "in_names": in_names, "out_names": out_names,
              "zero_outs": zero_outs, "n_params": n_params}
    _cached["runner"] = runner
    return runner


def kernel(**inputs):
    in_maps, co = make_in_maps(**inputs)
    try:
        import jax
        r = _get_runner()
        concat_in = [np.concatenate([m[name] for m in in_maps], axis=0)
                     for name in r["in_names"]]
        args = [jax.device_put(a) for a in concat_in + r["zero_outs"]]
        outs = r["fn"](*args)
        res = np.asarray(outs[r["out_names"].index("out")])
        parts = np.split(res, NCORES, axis=0)
        return gather([{"out": p} for p in parts], co)
    except Exception:
        from concourse.bass_utils import run_bass_kernel_spmd
        nc = _get_nc()
        res = run_bass_kernel_spmd(nc, in_maps, core_ids=list(range(NCORES)))
        return gather(res.results, co)
